# revision 21
# baseline (speedup 1.0000x reference)
"""Cross-attention kernel for Trainium2, distributed over 8 NeuronCores.

Problem: B=4, Sk=4096, Sq=2048, d_model=1024, dims=64 (fp32 reference).

Sharding (hardcoded): core c -> (batch b = c//2, decoder half h = c%2).
Each core computes out[b, h*1024:(h+1)*1024, :] from enc[b] and its decoder
slice. No collectives.

Per-core dataflow (all layouts chosen so no large on-chip transposes are
needed):
  - Host pre-transposes/casts activations to bf16 and packs them so every
    big DMA reads contiguous 8KB-per-partition runs: encA [p, kp, h, c, k],
    decA [p, h, c, k] (d_model chunk c*128+p on partitions).
  - KV^T projection: lhsT = [Wv | Wk] [128d, 128], rhs = encT chunks
    -> psum [128, 512] where rows 0:64 = V^T, 64:128 = K^T. Full PE array.
    Both 512-column chunks of a 1024-column pair are projected under one
    weight load per d-chunk (the second matmul reuses the loaded weights
    via ldweights=False) to halve exposed LDWEIGHTS time.
  - V^T is evacuated into a tile whose row 64 is constant 1.0; PE transposes
    yield V-natural blocks [128k, 65] whose col 64 is the ones column ->
    AV lhsT directly, so the ones column accumulates the softmax denominator
    during the AV matmul (no separate row-sum pass).
  - Scores computed transposed: S^T[k,q] = (K Q^T) with K^T/q operands
    duplicated on both partition halves so the two matmuls of a k-block
    run CONCURRENTLY via PE row-tiling ((0,0) / (64,0)).
    exp(S^T) on ACT (PSUM->SBUF bf16) feeds the AV matmul as the moving
    operand. No max-subtraction (|scores| ~ N(0,1), exp is safe in fp32).
    The body is a software pipeline because Tile compiles a fixed
    per-engine execution order; in the early (HBM-stream-bound)
    iterations the ready AV work runs BEFORE the next kv projection so
    the PE isn't parked behind a DMA-gated matmul.
  - out^T [65, q] accumulated in PSUM over k blocks; the unnormalized
    [65, 1024] block (row 64 = softmax denominator) is stored in bf16 and
    the normalization + final transpose happen on the host.
  - Startup: DMA descriptor enqueue is split across the sync (enc),
    scalar (dec) and gpsimd (constants) queues, big streams first.
"""

import numpy as np
import ml_dtypes
from einops import rearrange as _re

import concourse.bass as bass
import concourse.bacc as bacc
import concourse.tile as tile
from concourse import mybir
from concourse._compat import with_exitstack
from concourse.bass_utils import run_bass_kernel_spmd
from concourse.masks import make_identity

BF16 = mybir.dt.bfloat16
F32 = mybir.dt.float32

B, SK, SQ_FULL, D, DIMS = 4, 4096, 2048, 1024, 64
N_CORES = 8
SQ = SQ_FULL * B // N_CORES  # 1024 decoder rows per core
DC = D // 128  # d_model chunks of 128
KPAIRS = SK // 1024  # 4 enc column pair-tiles
KBLKS = SK // 128  # 32 k blocks for attention


def _mm_noldw(nc, out, lhsT, rhs, start, stop, skip_group_check=False):
    """nc.tensor.matmul but with ldweights=False: reuse the stationary
    operand already in the PE array from the immediately preceding matmul
    (same lhsT). Skips the per-matmul LDWEIGHTS that bass always emits."""
    te = nc.tensor
    keep = {0}
    ifmap_ap = te.lower_ap(rhs.opt(keep), opt=False)
    weights_ap = te.lower_ap(lhsT.opt(keep), opt=False, for_matmul_weights=True)
    out_ap = te.lower_ap(out)

    def rup(s):
        for v in (32, 64, 128):
            if v >= s:
                return v

    ts = (rup(rhs.partition_size()), rup(out.partition_size()))
    tp = (lhsT.base_partition(), out.base_partition())
    return te.add_instruction(mybir.InstMatmult(
        name=te.bass.get_next_instruction_name(),
        replication_resolution=0, replication_shift_amnt=0,
        replication_num_rows=0,
        start_tensor_calc=start, stop_tensor_calc=stop,
        ins=[ifmap_ap, weights_ap], outs=[out_ap],
        perf_mode=None, is_transpose=None,
        ifmap_quant_offset=None, weights_quant_offset=None,
        bass_skip_group_check=skip_group_check,
        tile_position=tp, tile_size=ts,
        ldweights=False,
    ))


@with_exitstack
def _body(ctx, tc, encA, decA, wkv, wq, bv, bk, bq, out):
    nc = tc.nc

    singles = ctx.enter_context(tc.tile_pool(name="singles", bufs=1))
    loads = ctx.enter_context(tc.tile_pool(name="loads", bufs=1))
    ps_pool = ctx.enter_context(tc.tile_pool(name="ps", bufs=2, space="PSUM"))
    po_pool = ctx.enter_context(tc.tile_pool(name="po", bufs=2, space="PSUM"))
    at_pool = ctx.enter_context(tc.tile_pool(name="at", bufs=6))
    outs = ctx.enter_context(tc.tile_pool(name="outs", bufs=1))

    # --- activation loads first: their descriptor enqueue gates everything.
    esbs = []
    for kp in range(KPAIRS):
        e0 = loads.tile([128, 4, 1024], BF16, tag=f"esb{kp}a", name=f"esb{kp}a")
        e1 = loads.tile([128, 4, 1024], BF16, tag=f"esb{kp}b", name=f"esb{kp}b")
        esbs.append((e0, e1))
    dsb = loads.tile([128, DC, SQ], BF16, tag="dload")

    # pair 0 split into d-quarters so the first kv matmuls start early
    for h in range(2):
        for q in range(2):
            nc.sync.dma_start(
                out=esbs[0][h][:, 2 * q:2 * q + 2, :],
                in_=encA[:, 0, h, 2 * q:2 * q + 2, :],
            )
    for h in range(2):
        nc.scalar.dma_start(
            out=dsb[:, h * 4:(h + 1) * 4, :], in_=decA[:, h, :, :]
        )
    for kp in range(1, KPAIRS):
        for h in range(2):
            nc.sync.dma_start(out=esbs[kp][h], in_=encA[:, kp, h, :, :])

    # --- constants on the SWDGE/gpsimd queue ---
    wkv_sb = singles.tile([128, DC, 128], BF16)
    nc.gpsimd.dma_start(out=wkv_sb, in_=wkv)
    wq_sb = singles.tile([128, DC, 128], BF16)
    nc.gpsimd.dma_start(out=wq_sb, in_=wq)
    bv_sb = singles.tile([DIMS, 1], F32)
    nc.gpsimd.dma_start(out=bv_sb, in_=bv)
    bk_sb = singles.tile([DIMS, 1], F32)
    nc.gpsimd.dma_start(out=bk_sb, in_=bk)
    bq_sb = singles.tile([128, 1], F32)
    nc.gpsimd.dma_start(out=bq_sb, in_=bq)
    ident_bf = singles.tile([128, 128], BF16)
    make_identity(nc, ident_bf)

    # --- persistent activations ---
    kTd = singles.tile([128, SK], BF16)
    vTx = singles.tile([80, SK], BF16)
    nc.gpsimd.memset(vTx[64:80, :], 1.0)
    vnat = singles.tile([128, KBLKS, 80], BF16)
    qTd = singles.tile([128, SQ], BF16)

    po0 = po_pool.tile([DIMS + 1, 512], F32, tag="po")
    po1 = po_pool.tile([DIMS + 1, 512], F32, tag="po")
    pos = [po0, po1]

    def kv_adds(ck, pskv):
        sl = slice(ck * 512, (ck + 1) * 512)
        nc.vector.tensor_scalar_add(vTx[0:DIMS, sl], pskv[0:DIMS, :], bv_sb)
        nc.vector.tensor_scalar_add(kTd[0:DIMS, sl], pskv[DIMS:128, :], bk_sb)
        nc.vector.tensor_scalar_add(kTd[DIMS:128, sl], pskv[DIMS:128, :], bk_sb)

    def kv_trans(ck):
        for kb in range(ck * 4, (ck + 1) * 4):
            ptv = ps_pool.tile([128, DIMS + 1], BF16, tag="aux", name=f"ptv{kb % 2}")
            nc.tensor.transpose(
                ptv, vTx[0:DIMS + 1, kb * 128:(kb + 1) * 128],
                ident_bf[0:DIMS + 1, 0:DIMS + 1],
            )
            nc.vector.tensor_copy(vnat[:, kb, 0:DIMS + 1], ptv)

    def kv_evac_pair(kp, pa, pb):
        # both psum chunks are read by the DVE adds BEFORE any ptv tile
        # rotates onto their aux buffers
        kv_adds(2 * kp, pa)
        kv_adds(2 * kp + 1, pb)
        kv_trans(2 * kp)
        kv_trans(2 * kp + 1)

    # --- K/V projection for a 1024-column pair tile: one weight load per
    # d-chunk feeds both 512-column psum accumulations ---
    def kv_pair(kp):
        pa = ps_pool.tile([128, 512], F32, tag="aux", name="pskvA")
        pb = ps_pool.tile([128, 512], F32, tag="aux", name="pskvB")
        for d in range(DC):
            esb = esbs[kp][d // 4]
            nc.tensor.matmul(
                pa, lhsT=wkv_sb[:, d, :], rhs=esb[:, d % 4, 0:512],
                start=(d == 0), stop=(d == DC - 1),
            )
            _mm_noldw(
                nc, pb, wkv_sb[:, d, :], esb[:, d % 4, 512:1024],
                start=(d == 0), stop=(d == DC - 1),
            )
        kv_evac_pair(kp, pa, pb)

    # --- attention for one pair of k blocks ---
    at_tiles = {}

    def s_exp_group(kg):
        psses = []
        for kb in (2 * kg, 2 * kg + 1):
            pss = ps_pool.tile([128, 2, 512], F32, tag="ps", name=f"pss{kb % 2}")
            psses.append(pss)
            for j in range(2):
                hp = DIMS * j
                nc.tensor.matmul(
                    pss[:, j, :], lhsT=kTd[hp:hp + DIMS, kb * 128:(kb + 1) * 128],
                    rhs=qTd[hp:hp + DIMS, j * 512:(j + 1) * 512],
                    start=True, stop=True,
                )
        for i, kb in enumerate((2 * kg, 2 * kg + 1)):
            at = at_pool.tile([128, 2, 512], BF16, tag="at", name=f"at{kb % 4}")
            at_tiles[kb] = at
            nc.scalar.activation(at, psses[i], mybir.ActivationFunctionType.Exp)

    def av_group(kg):
        for kb in (2 * kg, 2 * kg + 1):
            at = at_tiles.pop(kb)
            nc.tensor.matmul(
                pos[0], lhsT=vnat[:, kb, 0:DIMS + 1], rhs=at[:, 0, :],
                start=(kb == 0), stop=(kb == KBLKS - 1),
            )
            _mm_noldw(
                nc, pos[1], vnat[:, kb, 0:DIMS + 1], at[:, 1, :],
                start=(kb == 0), stop=(kb == KBLKS - 1),
            )

    # --- prologue: kv pair 0 and the Q projection interleaved by d-chunk;
    # each wq weight load feeds both q-half matmuls ---
    pa0 = ps_pool.tile([128, 512], F32, tag="aux", name="pskvA")
    pb0 = ps_pool.tile([128, 512], F32, tag="aux", name="pskvB")
    psq = ps_pool.tile([128, 2, 512], F32, tag="ps", name="psq")
    for half in range(2):
        for d in range(half * 4, half * 4 + 4):
            esb = esbs[0][d // 4]
            nc.tensor.matmul(
                pa0, lhsT=wkv_sb[:, d, :], rhs=esb[:, d % 4, 0:512],
                start=(d == 0), stop=(d == DC - 1),
            )
            _mm_noldw(
                nc, pb0, wkv_sb[:, d, :], esb[:, d % 4, 512:1024],
                start=(d == 0), stop=(d == DC - 1),
            )
        for d in range(half * 4, half * 4 + 4):
            nc.tensor.matmul(
                psq[:, 0, :], lhsT=wq_sb[:, d, :], rhs=dsb[:, d, 0:512],
                start=(d == 0), stop=(d == DC - 1),
            )
            _mm_noldw(
                nc, psq[:, 1, :], wq_sb[:, d, :], dsb[:, d, 512:1024],
                start=(d == 0), stop=(d == DC - 1),
            )
    for j in range(2):
        nc.vector.tensor_scalar_add(qTd[:, j * 512:(j + 1) * 512], psq[:, j, :], bq_sb)
    kv_evac_pair(0, pa0, pb0)

    NCK = SK // 512
    for ck in range(NCK):
        s_exp_group(2 * ck)
        s_exp_group(2 * ck + 1)
        if ck in (1, 2):
            # early iterations are HBM-stream-bound: run the ready AV work
            # BEFORE stalling on the next enc pair projection
            av_group(2 * (ck - 1))
            av_group(2 * (ck - 1) + 1)
            if ck % 2 == 1:
                kv_pair((ck + 1) // 2)
        else:
            if ck % 2 == 1 and ck < NCK - 1:
                kv_pair((ck + 1) // 2)
            if ck > 0:
                av_group(2 * (ck - 1))
                av_group(2 * (ck - 1) + 1)
    av_group(2 * (NCK - 1))
    av_group(2 * (NCK - 1) + 1)

    # --- output: unnormalized [65, 1024] (row 64 = denominator) in bf16;
    # normalization + transpose happen on the host ---
    out_sb = outs.tile([DIMS + 1, 2, 512], BF16, tag="osb")
    for j in range(2):
        nc.vector.tensor_copy(out_sb[:, j, :], pos[j])
    nc.sync.dma_start(out=out, in_=out_sb)


_NC_CACHE = None


def _build():
    global _NC_CACHE
    if _NC_CACHE is not None:
        return _NC_CACHE
    nc = bacc.Bacc(
        "TRN2", target_bir_lowering=False, debug=False,
        enable_asserts=True, num_devices=N_CORES,
    )
    encA = nc.dram_tensor("encA", [128, KPAIRS, 2, 4, 1024], BF16,
                          kind="ExternalInput").ap()
    decA = nc.dram_tensor("decA", [128, 2, 4, SQ], BF16,
                          kind="ExternalInput").ap()
    wkv = nc.dram_tensor("wkv", [128, DC, 128], BF16, kind="ExternalInput").ap()
    wq = nc.dram_tensor("wq", [128, DC, 128], BF16, kind="ExternalInput").ap()
    bv = nc.dram_tensor("bv", [DIMS, 1], F32, kind="ExternalInput").ap()
    bk = nc.dram_tensor("bk", [DIMS, 1], F32, kind="ExternalInput").ap()
    bq = nc.dram_tensor("bq", [128, 1], F32, kind="ExternalInput").ap()
    out = nc.dram_tensor("out", [DIMS + 1, 2, 512], BF16,
                         kind="ExternalOutput").ap()
    with tile.TileContext(nc) as tc:
        _body(tc, encA, decA, wkv, wq, bv, bk, bq, out)
    nc.compile()
    _NC_CACHE = nc
    return nc


def make_in_maps(**inputs):
    bf16 = ml_dtypes.bfloat16
    enc = np.asarray(inputs["encoder_output"])
    dec = np.asarray(inputs["decoder"])
    scale = DIMS ** -0.5
    wq1 = np.asarray(inputs["Wq"]) * scale
    wq_s = np.concatenate([wq1, wq1], axis=1).astype(bf16)
    wq_s = _re(wq_s, "(c p) m -> p c m", p=128)
    bq1 = (np.asarray(inputs["bq"]) * scale).astype(np.float32).reshape(DIMS, 1)
    bq_s = np.concatenate([bq1, bq1], axis=0)
    wkv = np.concatenate(
        [np.asarray(inputs["Wv"]), np.asarray(inputs["Wk"])], axis=1
    ).astype(bf16)
    wkv = _re(wkv, "(c p) m -> p c m", p=128)
    bv = np.asarray(inputs["bv"]).astype(np.float32).reshape(DIMS, 1)
    bk = np.asarray(inputs["bk"]).astype(np.float32).reshape(DIMS, 1)
    in_maps = []
    for c in range(N_CORES):
        b, h = divmod(c, 2)
        encA = _re(np.ascontiguousarray(enc[b].T).astype(bf16),
                   "(h c p) (kp k) -> p kp h c k", h=2, c=4, p=128, k=1024)
        decT = np.ascontiguousarray(dec[b, h * SQ:(h + 1) * SQ, :].T).astype(bf16)
        decA = _re(decT, "(h c p) k -> p h c k", h=2, c=4, p=128)
        in_maps.append({
            "encA": np.ascontiguousarray(encA),
            "decA": np.ascontiguousarray(decA),
            "wkv": wkv, "wq": wq_s, "bv": bv, "bk": bk, "bq": bq_s,
        })
    return in_maps


def assemble(results):
    out = np.zeros((B, SQ_FULL, DIMS), np.float32)
    for c in range(N_CORES):
        b, h = divmod(c, 2)
        o = results[c]["out"].reshape(DIMS + 1, SQ).astype(np.float32)
        out[b, h * SQ:(h + 1) * SQ] = (o[0:DIMS] / o[DIMS:DIMS + 1]).T
    return out


def kernel(**inputs) -> np.ndarray:
    nc = _build()
    in_maps = make_in_maps(**inputs)
    res = run_bass_kernel_spmd(nc, in_maps, core_ids=list(range(N_CORES)))
    return assemble(res.results)


# revision 26
# speedup vs baseline: 1.0790x; 1.0790x over previous
"""Cross-attention kernel for Trainium2, distributed over 8 NeuronCores.

Problem: B=4, Sk=4096, Sq=2048, d_model=1024, dims=64 (fp32 reference).

Sharding (hardcoded): core c -> (batch b = c//2, decoder half h = c%2).
Each core computes out[b, h*1024:(h+1)*1024, :] from enc[b] and its decoder
slice. No collectives.

Per-core dataflow (all layouts chosen so no large on-chip transposes are
needed):
  - Host pre-transposes/casts activations to bf16 and packs them so every
    big DMA reads contiguous 8KB-per-partition runs: encA [p, kp, h, c, k],
    decA [p, h, c, k] (d_model chunk c*128+p on partitions).
  - KV^T projection: lhsT = [Wv | Wk] [128d, 128], rhs = encT chunks
    -> psum [128, 512] where rows 0:64 = V^T, 64:128 = K^T. Full PE array.
    Both 512-column chunks of a 1024-column pair are projected under one
    weight load per d-chunk (the second matmul reuses the loaded weights
    via ldweights=False) to halve exposed LDWEIGHTS time.
  - V^T is evacuated into a tile whose row 64 is constant 1.0; PE transposes
    yield V-natural blocks [128k, 65] whose col 64 is the ones column ->
    AV lhsT directly, so the ones column accumulates the softmax denominator
    during the AV matmul (no separate row-sum pass).
  - Scores computed transposed: S^T[k,q] = (K Q^T) with K^T/q operands
    duplicated on both partition halves so the two matmuls of a k-block
    run CONCURRENTLY via PE row-tiling ((0,0) / (64,0)).
    exp(S^T) on ACT (PSUM->SBUF bf16) feeds the AV matmul as the moving
    operand. No max-subtraction (|scores| ~ N(0,1), exp is safe in fp32).
    The body is a software pipeline because Tile compiles a fixed
    per-engine execution order; in the early (HBM-stream-bound)
    iterations the ready AV work runs BEFORE the next kv projection so
    the PE isn't parked behind a DMA-gated matmul.
  - out^T [65, q] accumulated in PSUM over k blocks; the unnormalized
    [65, 1024] block (row 64 = softmax denominator) is stored in bf16 and
    the normalization + final transpose happen on the host.
  - Startup: DMA descriptor enqueue is split across the sync (enc),
    scalar (dec) and gpsimd (constants) queues, big streams first.
"""

import numpy as np
import ml_dtypes
from einops import rearrange as _re

import concourse.bass as bass
import concourse.bacc as bacc
import concourse.tile as tile
from concourse import mybir
from concourse._compat import with_exitstack
from concourse.bass_utils import run_bass_kernel_spmd
from concourse.masks import make_identity

BF16 = mybir.dt.bfloat16
F32 = mybir.dt.float32

B, SK, SQ_FULL, D, DIMS = 4, 4096, 2048, 1024, 64
N_CORES = 8
SQ = SQ_FULL * B // N_CORES  # 1024 decoder rows per core
DC = D // 128  # d_model chunks of 128
KPAIRS = SK // 1024  # 4 enc column pair-tiles
KBLKS = SK // 128  # 32 k blocks for attention


@with_exitstack
def _body(ctx, tc, encA, decA, wkv, wq, bv, bk, bq, out):
    nc = tc.nc

    singles = ctx.enter_context(tc.tile_pool(name="singles", bufs=1))
    loads = ctx.enter_context(tc.tile_pool(name="loads", bufs=1))
    ps_pool = ctx.enter_context(tc.tile_pool(name="ps", bufs=2, space="PSUM"))
    po_pool = ctx.enter_context(tc.tile_pool(name="po", bufs=2, space="PSUM"))
    at_pool = ctx.enter_context(tc.tile_pool(name="at", bufs=6))
    outs = ctx.enter_context(tc.tile_pool(name="outs", bufs=1))

    # --- activation loads first: their descriptor enqueue gates everything.
    esbs = []
    for kp in range(KPAIRS):
        e0 = loads.tile([128, 4, 1024], BF16, tag=f"esb{kp}a", name=f"esb{kp}a")
        e1 = loads.tile([128, 4, 1024], BF16, tag=f"esb{kp}b", name=f"esb{kp}b")
        esbs.append((e0, e1))
    dsb = loads.tile([128, DC, SQ], BF16, tag="dload")

    # pair 0 split into d-quarters so the first kv matmuls start early
    for h in range(2):
        for q in range(2):
            nc.sync.dma_start(
                out=esbs[0][h][:, 2 * q:2 * q + 2, :],
                in_=encA[:, 0, h, 2 * q:2 * q + 2, :],
            )
    for h in range(2):
        nc.scalar.dma_start(
            out=dsb[:, h * 4:(h + 1) * 4, :], in_=decA[:, h, :, :]
        )
    for kp in range(1, KPAIRS):
        for h in range(2):
            nc.sync.dma_start(out=esbs[kp][h], in_=encA[:, kp, h, :, :])

    # --- constants on the SWDGE/gpsimd queue ---
    wkv_sb = singles.tile([128, DC, 128], BF16)
    nc.gpsimd.dma_start(out=wkv_sb, in_=wkv)
    wq_sb = singles.tile([128, DC, 128], BF16)
    nc.gpsimd.dma_start(out=wq_sb, in_=wq)
    bv_sb = singles.tile([DIMS, 1], F32)
    nc.gpsimd.dma_start(out=bv_sb, in_=bv)
    bk_sb = singles.tile([DIMS, 1], F32)
    nc.gpsimd.dma_start(out=bk_sb, in_=bk)
    bq_sb = singles.tile([128, 1], F32)
    nc.gpsimd.dma_start(out=bq_sb, in_=bq)
    ident_bf = singles.tile([128, 128], BF16)
    make_identity(nc, ident_bf)

    # --- persistent activations ---
    kTd = singles.tile([128, SK], BF16)
    vTx = singles.tile([80, SK], BF16)
    nc.gpsimd.memset(vTx[64:80, :], 1.0)
    vnat = singles.tile([128, KBLKS, 80], BF16)
    qTd = singles.tile([128, SQ], BF16)

    po0 = po_pool.tile([DIMS + 1, 512], F32, tag="po")
    po1 = po_pool.tile([DIMS + 1, 512], F32, tag="po")
    pos = [po0, po1]

    def kv_adds(ck, pskv):
        sl = slice(ck * 512, (ck + 1) * 512)
        nc.vector.tensor_scalar_add(vTx[0:DIMS, sl], pskv[0:DIMS, :], bv_sb)
        nc.vector.tensor_scalar_add(kTd[0:DIMS, sl], pskv[DIMS:128, :], bk_sb)
        nc.vector.tensor_scalar_add(kTd[DIMS:128, sl], pskv[DIMS:128, :], bk_sb)

    def kv_trans(ck):
        for kb in range(ck * 4, (ck + 1) * 4):
            ptv = ps_pool.tile([128, DIMS + 1], BF16, tag="aux", name=f"ptv{kb % 2}")
            nc.tensor.transpose(
                ptv, vTx[0:DIMS + 1, kb * 128:(kb + 1) * 128],
                ident_bf[0:DIMS + 1, 0:DIMS + 1],
            )
            nc.vector.tensor_copy(vnat[:, kb, 0:DIMS + 1], ptv)

    # --- K/V projection + V transpose for one 512-column chunk (4 k blocks)
    def kv_chunk(ck):
        pskv = ps_pool.tile([128, 512], F32, tag="aux", name=f"pskv{ck % 2}")
        for d in range(DC):
            esb = esbs[ck // 2][d // 4]
            nc.tensor.matmul(
                pskv, lhsT=wkv_sb[:, d, :],
                rhs=esb[:, d % 4, (ck % 2) * 512:(ck % 2 + 1) * 512],
                start=(d == 0), stop=(d == DC - 1),
            )
        kv_adds(ck, pskv)
        kv_trans(ck)

    # --- attention for one pair of k blocks ---
    at_tiles = {}

    def s_exp_group(kg):
        psses = []
        for kb in (2 * kg, 2 * kg + 1):
            pss = ps_pool.tile([128, 2, 512], F32, tag="ps", name=f"pss{kb % 2}")
            psses.append(pss)
            for j in range(2):
                hp = DIMS * j
                nc.tensor.matmul(
                    pss[:, j, :], lhsT=kTd[hp:hp + DIMS, kb * 128:(kb + 1) * 128],
                    rhs=qTd[hp:hp + DIMS, j * 512:(j + 1) * 512],
                    start=True, stop=True,
                )
        for i, kb in enumerate((2 * kg, 2 * kg + 1)):
            at = at_pool.tile([128, 2, 512], BF16, tag="at", name=f"at{kb % 4}")
            at_tiles[kb] = at
            nc.scalar.activation(at, psses[i], mybir.ActivationFunctionType.Exp)

    def av_group(kg):
        for kb in (2 * kg, 2 * kg + 1):
            at = at_tiles.pop(kb)
            for j in range(2):
                nc.tensor.matmul(
                    pos[j], lhsT=vnat[:, kb, 0:DIMS + 1], rhs=at[:, j, :],
                    start=(kb == 0), stop=(kb == KBLKS - 1),
                )

    # --- prologue: a short junk warmup fits inside the initial DMA wait
    # (nothing real is runnable before ~8us), then kv chunk 0 and the Q
    # projection interleaved by d-quarter to match the DMA arrival order ---
    junk = singles.tile([128, 256], BF16)
    nc.vector.memset(junk, 0.0)
    junk_ps = ps_pool.tile([128, 2, 512], F32, tag="ps", name="junkps")
    for _ in range(10):
        nc.tensor.matmul(
            junk_ps[:, 0, 0:256], lhsT=junk[:, 0:128], rhs=junk,
            start=True, stop=True,
        )
    pskv0 = ps_pool.tile([128, 512], F32, tag="aux", name="pskv0p")
    psq = ps_pool.tile([128, 2, 512], F32, tag="ps", name="psq")
    for half in range(2):
        for d in range(half * 4, half * 4 + 4):
            nc.tensor.matmul(
                pskv0, lhsT=wkv_sb[:, d, :],
                rhs=esbs[0][d // 4][:, d % 4, 0:512],
                start=(d == 0), stop=(d == DC - 1),
            )
        for d in range(half * 4, half * 4 + 4):
            for j in range(2):
                nc.tensor.matmul(
                    psq[:, j, :], lhsT=wq_sb[:, d, :],
                    rhs=dsb[:, d, j * 512:(j + 1) * 512],
                    start=(d == 0), stop=(d == DC - 1),
                )
    for j in range(2):
        nc.vector.tensor_scalar_add(qTd[:, j * 512:(j + 1) * 512], psq[:, j, :], bq_sb)
    kv_adds(0, pskv0)
    kv_trans(0)

    NCK = SK // 512
    for ck in range(NCK):
        s_exp_group(2 * ck)
        s_exp_group(2 * ck + 1)
        if ck in (1, 2):
            # early iterations are HBM-stream-bound: run the ready AV work
            # BEFORE stalling on the next enc chunk projection
            av_group(2 * (ck - 1))
            av_group(2 * (ck - 1) + 1)
            kv_chunk(ck + 1)
        else:
            if ck + 1 < NCK:
                kv_chunk(ck + 1)
            if ck > 0:
                av_group(2 * (ck - 1))
                av_group(2 * (ck - 1) + 1)
    av_group(2 * (NCK - 1))
    av_group(2 * (NCK - 1) + 1)

    # --- output: unnormalized [65, 1024] (row 64 = denominator) in bf16;
    # normalization + transpose happen on the host ---
    out_sb = outs.tile([DIMS + 1, 2, 512], BF16, tag="osb")
    for j in range(2):
        nc.vector.tensor_copy(out_sb[:, j, :], pos[j])
    nc.sync.dma_start(out=out, in_=out_sb)


_NC_CACHE = None


def _build():
    global _NC_CACHE
    if _NC_CACHE is not None:
        return _NC_CACHE
    nc = bacc.Bacc(
        "TRN2", target_bir_lowering=False, debug=False,
        enable_asserts=True, num_devices=N_CORES,
    )
    encA = nc.dram_tensor("encA", [128, KPAIRS, 2, 4, 1024], BF16,
                          kind="ExternalInput").ap()
    decA = nc.dram_tensor("decA", [128, 2, 4, SQ], BF16,
                          kind="ExternalInput").ap()
    wkv = nc.dram_tensor("wkv", [128, DC, 128], BF16, kind="ExternalInput").ap()
    wq = nc.dram_tensor("wq", [128, DC, 128], BF16, kind="ExternalInput").ap()
    bv = nc.dram_tensor("bv", [DIMS, 1], F32, kind="ExternalInput").ap()
    bk = nc.dram_tensor("bk", [DIMS, 1], F32, kind="ExternalInput").ap()
    bq = nc.dram_tensor("bq", [128, 1], F32, kind="ExternalInput").ap()
    out = nc.dram_tensor("out", [DIMS + 1, 2, 512], BF16,
                         kind="ExternalOutput").ap()
    with tile.TileContext(nc) as tc:
        _body(tc, encA, decA, wkv, wq, bv, bk, bq, out)
    nc.compile()
    _NC_CACHE = nc
    return nc


def make_in_maps(**inputs):
    bf16 = ml_dtypes.bfloat16
    enc = np.asarray(inputs["encoder_output"])
    dec = np.asarray(inputs["decoder"])
    scale = DIMS ** -0.5
    wq1 = np.asarray(inputs["Wq"]) * scale
    wq_s = np.concatenate([wq1, wq1], axis=1).astype(bf16)
    wq_s = _re(wq_s, "(c p) m -> p c m", p=128)
    bq1 = (np.asarray(inputs["bq"]) * scale).astype(np.float32).reshape(DIMS, 1)
    bq_s = np.concatenate([bq1, bq1], axis=0)
    wkv = np.concatenate(
        [np.asarray(inputs["Wv"]), np.asarray(inputs["Wk"])], axis=1
    ).astype(bf16)
    wkv = _re(wkv, "(c p) m -> p c m", p=128)
    bv = np.asarray(inputs["bv"]).astype(np.float32).reshape(DIMS, 1)
    bk = np.asarray(inputs["bk"]).astype(np.float32).reshape(DIMS, 1)
    in_maps = []
    for c in range(N_CORES):
        b, h = divmod(c, 2)
        encA = _re(np.ascontiguousarray(enc[b].T).astype(bf16),
                   "(h c p) (kp k) -> p kp h c k", h=2, c=4, p=128, k=1024)
        decT = np.ascontiguousarray(dec[b, h * SQ:(h + 1) * SQ, :].T).astype(bf16)
        decA = _re(decT, "(h c p) k -> p h c k", h=2, c=4, p=128)
        in_maps.append({
            "encA": np.ascontiguousarray(encA),
            "decA": np.ascontiguousarray(decA),
            "wkv": wkv, "wq": wq_s, "bv": bv, "bk": bk, "bq": bq_s,
        })
    return in_maps


def assemble(results):
    out = np.zeros((B, SQ_FULL, DIMS), np.float32)
    for c in range(N_CORES):
        b, h = divmod(c, 2)
        o = results[c]["out"].reshape(DIMS + 1, SQ).astype(np.float32)
        out[b, h * SQ:(h + 1) * SQ] = (o[0:DIMS] / o[DIMS:DIMS + 1]).T
    return out


def kernel(**inputs) -> np.ndarray:
    nc = _build()
    in_maps = make_in_maps(**inputs)
    res = run_bass_kernel_spmd(nc, in_maps, core_ids=list(range(N_CORES)))
    return assemble(res.results)


# revision 27
# speedup vs baseline: 1.0833x; 1.0040x over previous
"""Cross-attention kernel for Trainium2, distributed over 8 NeuronCores.

Problem: B=4, Sk=4096, Sq=2048, d_model=1024, dims=64 (fp32 reference).

Sharding (hardcoded): core c -> (batch b = c//2, decoder half h = c%2).
Each core computes out[b, h*1024:(h+1)*1024, :] from enc[b] and its decoder
slice. No collectives.

Per-core dataflow (all layouts chosen so no large on-chip transposes are
needed):
  - Host pre-transposes/casts activations to bf16 and packs them so every
    big DMA reads contiguous 8KB-per-partition runs: encA [p, kp, h, c, k],
    decA [p, h, c, k] (d_model chunk c*128+p on partitions).
  - KV^T projection: lhsT = [Wv | Wk] [128d, 128], rhs = encT chunks
    -> psum [128, 512] where rows 0:64 = V^T, 64:128 = K^T. Full PE array.
    Both 512-column chunks of a 1024-column pair are projected under one
    weight load per d-chunk (the second matmul reuses the loaded weights
    via ldweights=False) to halve exposed LDWEIGHTS time.
  - V^T is evacuated into a tile whose row 64 is constant 1.0; PE transposes
    yield V-natural blocks [128k, 65] whose col 64 is the ones column ->
    AV lhsT directly, so the ones column accumulates the softmax denominator
    during the AV matmul (no separate row-sum pass).
  - Scores computed transposed: S^T[k,q] = (K Q^T) with K^T/q operands
    duplicated on both partition halves so the two matmuls of a k-block
    run CONCURRENTLY via PE row-tiling ((0,0) / (64,0)).
    exp(S^T) on ACT (PSUM->SBUF bf16) feeds the AV matmul as the moving
    operand. No max-subtraction (|scores| ~ N(0,1), exp is safe in fp32).
    The body is a software pipeline because Tile compiles a fixed
    per-engine execution order; in the early (HBM-stream-bound)
    iterations the ready AV work runs BEFORE the next kv projection so
    the PE isn't parked behind a DMA-gated matmul.
  - out^T [65, q] accumulated in PSUM over k blocks; the unnormalized
    [65, 1024] block (row 64 = softmax denominator) is stored in bf16 and
    the normalization + final transpose happen on the host.
  - Startup: DMA descriptor enqueue is split across the sync (enc),
    scalar (dec) and gpsimd (constants) queues, big streams first.
"""

import numpy as np
import ml_dtypes
from einops import rearrange as _re

import concourse.bass as bass
import concourse.bacc as bacc
import concourse.tile as tile
from concourse import mybir
from concourse._compat import with_exitstack
from concourse.bass_utils import run_bass_kernel_spmd
from concourse.masks import make_identity

BF16 = mybir.dt.bfloat16
F32 = mybir.dt.float32

B, SK, SQ_FULL, D, DIMS = 4, 4096, 2048, 1024, 64
N_CORES = 8
SQ = SQ_FULL * B // N_CORES  # 1024 decoder rows per core
DC = D // 128  # d_model chunks of 128
KPAIRS = SK // 1024  # 4 enc column pair-tiles
KBLKS = SK // 128  # 32 k blocks for attention


@with_exitstack
def _body(ctx, tc, encA, decA, wkv, wq, bv, bk, bq, out):
    nc = tc.nc

    singles = ctx.enter_context(tc.tile_pool(name="singles", bufs=1))
    loads = ctx.enter_context(tc.tile_pool(name="loads", bufs=1))
    ps_pool = ctx.enter_context(tc.tile_pool(name="ps", bufs=2, space="PSUM"))
    po_pool = ctx.enter_context(tc.tile_pool(name="po", bufs=2, space="PSUM"))
    at_pool = ctx.enter_context(tc.tile_pool(name="at", bufs=6))
    outs = ctx.enter_context(tc.tile_pool(name="outs", bufs=1))

    # --- activation loads first: their descriptor enqueue gates everything.
    esbs = []
    for kp in range(KPAIRS):
        e0 = loads.tile([128, 4, 1024], BF16, tag=f"esb{kp}a", name=f"esb{kp}a")
        e1 = loads.tile([128, 4, 1024], BF16, tag=f"esb{kp}b", name=f"esb{kp}b")
        esbs.append((e0, e1))
    dsb = loads.tile([128, DC, SQ], BF16, tag="dload")

    # pair 0 split into d-quarters so the first kv matmuls start early
    for h in range(2):
        for q in range(2):
            nc.sync.dma_start(
                out=esbs[0][h][:, 2 * q:2 * q + 2, :],
                in_=encA[:, 0, h, 2 * q:2 * q + 2, :],
            )
    for h in range(2):
        nc.scalar.dma_start(
            out=dsb[:, h * 4:(h + 1) * 4, :], in_=decA[:, h, :, :]
        )
    for kp in range(1, KPAIRS):
        for h in range(2):
            nc.sync.dma_start(out=esbs[kp][h], in_=encA[:, kp, h, :, :])

    # --- constants on the SWDGE/gpsimd queue ---
    wkv_sb = singles.tile([128, DC, 128], BF16)
    nc.gpsimd.dma_start(out=wkv_sb, in_=wkv)
    wq_sb = singles.tile([128, DC, 128], BF16)
    nc.gpsimd.dma_start(out=wq_sb, in_=wq)
    bv_sb = singles.tile([DIMS, 1], F32)
    nc.gpsimd.dma_start(out=bv_sb, in_=bv)
    bk_sb = singles.tile([DIMS, 1], F32)
    nc.gpsimd.dma_start(out=bk_sb, in_=bk)
    bq_sb = singles.tile([128, 1], F32)
    nc.gpsimd.dma_start(out=bq_sb, in_=bq)
    ident_bf = singles.tile([128, 128], BF16)
    make_identity(nc, ident_bf)

    # --- persistent activations ---
    kTd = singles.tile([128, SK], BF16)
    vTx = singles.tile([80, SK], BF16)
    nc.gpsimd.memset(vTx[64:80, :], 1.0)
    vnat = singles.tile([128, KBLKS, 80], BF16)
    qTd = singles.tile([128, SQ], BF16)

    po0 = po_pool.tile([DIMS + 1, 512], F32, tag="po")
    po1 = po_pool.tile([DIMS + 1, 512], F32, tag="po")
    pos = [po0, po1]

    def kv_adds(ck, pskv):
        sl = slice(ck * 512, (ck + 1) * 512)
        nc.vector.tensor_scalar_add(vTx[0:DIMS, sl], pskv[0:DIMS, :], bv_sb)
        nc.vector.tensor_scalar_add(kTd[0:DIMS, sl], pskv[DIMS:128, :], bk_sb)
        nc.vector.tensor_scalar_add(kTd[DIMS:128, sl], pskv[DIMS:128, :], bk_sb)

    def kv_trans(ck):
        for kb in range(ck * 4, (ck + 1) * 4):
            ptv = ps_pool.tile([128, DIMS + 1], BF16, tag="aux", name=f"ptv{kb % 2}")
            nc.tensor.transpose(
                ptv, vTx[0:DIMS + 1, kb * 128:(kb + 1) * 128],
                ident_bf[0:DIMS + 1, 0:DIMS + 1],
            )
            nc.vector.tensor_copy(vnat[:, kb, 0:DIMS + 1], ptv)

    # --- K/V projection + V transpose for one 512-column chunk (4 k blocks)
    def kv_chunk(ck):
        pskv = ps_pool.tile([128, 512], F32, tag="aux", name=f"pskv{ck % 2}")
        for d in range(DC):
            esb = esbs[ck // 2][d // 4]
            nc.tensor.matmul(
                pskv, lhsT=wkv_sb[:, d, :],
                rhs=esb[:, d % 4, (ck % 2) * 512:(ck % 2 + 1) * 512],
                start=(d == 0), stop=(d == DC - 1),
            )
        kv_adds(ck, pskv)
        kv_trans(ck)

    # --- attention for one pair of k blocks ---
    at_tiles = {}

    def s_exp_group(kg):
        psses = []
        for kb in (2 * kg, 2 * kg + 1):
            pss = ps_pool.tile([128, 2, 512], F32, tag="ps", name=f"pss{kb % 2}")
            psses.append(pss)
            for j in range(2):
                hp = DIMS * j
                nc.tensor.matmul(
                    pss[:, j, :], lhsT=kTd[hp:hp + DIMS, kb * 128:(kb + 1) * 128],
                    rhs=qTd[hp:hp + DIMS, j * 512:(j + 1) * 512],
                    start=True, stop=True,
                )
        for i, kb in enumerate((2 * kg, 2 * kg + 1)):
            at = at_pool.tile([128, 2, 512], BF16, tag="at", name=f"at{kb % 4}")
            at_tiles[kb] = at
            nc.scalar.activation(at, psses[i], mybir.ActivationFunctionType.Exp)

    def av_group(kg):
        for kb in (2 * kg, 2 * kg + 1):
            at = at_tiles.pop(kb)
            for j in range(2):
                nc.tensor.matmul(
                    pos[j], lhsT=vnat[:, kb, 0:DIMS + 1], rhs=at[:, j, :],
                    start=(kb == 0), stop=(kb == KBLKS - 1),
                )

    # --- prologue: a short junk warmup fits inside the initial DMA wait
    # (nothing real is runnable before ~8us), then kv chunk 0 and the Q
    # projection interleaved by d-quarter to match the DMA arrival order ---
    pskv0 = ps_pool.tile([128, 512], F32, tag="aux", name="pskv0p")
    psq = ps_pool.tile([128, 2, 512], F32, tag="ps", name="psq")
    for half in range(2):
        for d in range(half * 4, half * 4 + 4):
            nc.tensor.matmul(
                pskv0, lhsT=wkv_sb[:, d, :],
                rhs=esbs[0][d // 4][:, d % 4, 0:512],
                start=(d == 0), stop=(d == DC - 1),
            )
        for d in range(half * 4, half * 4 + 4):
            for j in range(2):
                nc.tensor.matmul(
                    psq[:, j, :], lhsT=wq_sb[:, d, :],
                    rhs=dsb[:, d, j * 512:(j + 1) * 512],
                    start=(d == 0), stop=(d == DC - 1),
                )
    for j in range(2):
        nc.vector.tensor_scalar_add(qTd[:, j * 512:(j + 1) * 512], psq[:, j, :], bq_sb)
    kv_adds(0, pskv0)
    kv_trans(0)

    NCK = SK // 512
    for ck in range(NCK):
        s_exp_group(2 * ck)
        s_exp_group(2 * ck + 1)
        if ck in (1, 2):
            # early iterations are HBM-stream-bound: run the ready AV work
            # BEFORE stalling on the next enc chunk projection
            av_group(2 * (ck - 1))
            av_group(2 * (ck - 1) + 1)
            kv_chunk(ck + 1)
        else:
            if ck + 1 < NCK:
                kv_chunk(ck + 1)
            if ck > 0:
                av_group(2 * (ck - 1))
                av_group(2 * (ck - 1) + 1)
    av_group(2 * (NCK - 1))
    av_group(2 * (NCK - 1) + 1)

    # --- output: unnormalized [65, 1024] (row 64 = denominator) in bf16;
    # normalization + transpose happen on the host ---
    out_sb = outs.tile([DIMS + 1, 2, 512], BF16, tag="osb")
    for j in range(2):
        nc.vector.tensor_copy(out_sb[:, j, :], pos[j])
    nc.sync.dma_start(out=out, in_=out_sb)


_NC_CACHE = None


def _build():
    global _NC_CACHE
    if _NC_CACHE is not None:
        return _NC_CACHE
    nc = bacc.Bacc(
        "TRN2", target_bir_lowering=False, debug=False,
        enable_asserts=True, num_devices=N_CORES,
    )
    encA = nc.dram_tensor("encA", [128, KPAIRS, 2, 4, 1024], BF16,
                          kind="ExternalInput").ap()
    decA = nc.dram_tensor("decA", [128, 2, 4, SQ], BF16,
                          kind="ExternalInput").ap()
    wkv = nc.dram_tensor("wkv", [128, DC, 128], BF16, kind="ExternalInput").ap()
    wq = nc.dram_tensor("wq", [128, DC, 128], BF16, kind="ExternalInput").ap()
    bv = nc.dram_tensor("bv", [DIMS, 1], F32, kind="ExternalInput").ap()
    bk = nc.dram_tensor("bk", [DIMS, 1], F32, kind="ExternalInput").ap()
    bq = nc.dram_tensor("bq", [128, 1], F32, kind="ExternalInput").ap()
    out = nc.dram_tensor("out", [DIMS + 1, 2, 512], BF16,
                         kind="ExternalOutput").ap()
    with tile.TileContext(nc) as tc:
        _body(tc, encA, decA, wkv, wq, bv, bk, bq, out)
    nc.compile()
    _NC_CACHE = nc
    return nc


def make_in_maps(**inputs):
    bf16 = ml_dtypes.bfloat16
    enc = np.asarray(inputs["encoder_output"])
    dec = np.asarray(inputs["decoder"])
    scale = DIMS ** -0.5
    wq1 = np.asarray(inputs["Wq"]) * scale
    wq_s = np.concatenate([wq1, wq1], axis=1).astype(bf16)
    wq_s = _re(wq_s, "(c p) m -> p c m", p=128)
    bq1 = (np.asarray(inputs["bq"]) * scale).astype(np.float32).reshape(DIMS, 1)
    bq_s = np.concatenate([bq1, bq1], axis=0)
    wkv = np.concatenate(
        [np.asarray(inputs["Wv"]), np.asarray(inputs["Wk"])], axis=1
    ).astype(bf16)
    wkv = _re(wkv, "(c p) m -> p c m", p=128)
    bv = np.asarray(inputs["bv"]).astype(np.float32).reshape(DIMS, 1)
    bk = np.asarray(inputs["bk"]).astype(np.float32).reshape(DIMS, 1)
    in_maps = []
    for c in range(N_CORES):
        b, h = divmod(c, 2)
        encA = _re(np.ascontiguousarray(enc[b].T).astype(bf16),
                   "(h c p) (kp k) -> p kp h c k", h=2, c=4, p=128, k=1024)
        decT = np.ascontiguousarray(dec[b, h * SQ:(h + 1) * SQ, :].T).astype(bf16)
        decA = _re(decT, "(h c p) k -> p h c k", h=2, c=4, p=128)
        in_maps.append({
            "encA": np.ascontiguousarray(encA),
            "decA": np.ascontiguousarray(decA),
            "wkv": wkv, "wq": wq_s, "bv": bv, "bk": bk, "bq": bq_s,
        })
    return in_maps


def assemble(results):
    out = np.zeros((B, SQ_FULL, DIMS), np.float32)
    for c in range(N_CORES):
        b, h = divmod(c, 2)
        o = results[c]["out"].reshape(DIMS + 1, SQ).astype(np.float32)
        out[b, h * SQ:(h + 1) * SQ] = (o[0:DIMS] / o[DIMS:DIMS + 1]).T
    return out


def kernel(**inputs) -> np.ndarray:
    nc = _build()
    in_maps = make_in_maps(**inputs)
    res = run_bass_kernel_spmd(nc, in_maps, core_ids=list(range(N_CORES)))
    return assemble(res.results)
